# revision 11
# baseline (speedup 1.0000x reference)
"""CorrelationHead Trainium2 kernel — V2 (parity-class packed fc1).

Math: corr features reduce to per-RoI Gram G[ij,kl] = sum_c x1[c,ij]*x2[c,kl];
only (ij,kl) with matching (i,k) and (j,l) parity are used by fc1 (625 of
2401). Spatial positions are CLASS-SORTED on host (parity classes A=16/B=12/
C=12/D=9 for both ij and kl), so the Gram in SBUF is [49 ij', 49 kl', b] with
class-contiguous ranges. SBUF->SBUF DMAs stack (slot,kl) pairs onto partitions
(7 chunks of <=128 rows = 625 valid pairs) and fc1 becomes 7 accumulating
matmuls per rep-chunk (K=128/72/81) instead of 49 K=49 matmuls.

fc1/fc2 computed transposed (psFT[r, b]) so ReLU bias rides the ACT bias port
(per-partition = per-rep) and no PE transposes are needed anywhere.

Sharding: pure data-parallel, 128 RoIs per core, weights replicated.
"""

import os
from contextlib import ExitStack

import numpy as np
import ml_dtypes

import concourse.bass as bass
import concourse.mybir as mybir
from concourse.bass_utils import run_bass_kernel_spmd

P = 16
H = 7
C = 256
B = 1024
REP = 1024
HW = H * H
N_CORES = 8
BL = B // N_CORES          # 128 RoIs per core
NG = 8                     # RoIs per psG group
NGROUP = BL // NG          # 16
NQ = 4                     # x DMA quarters
QW = BL // NQ * HW         # 1568 cols per quarter

F32 = mybir.dt.float32
BF16 = mybir.dt.bfloat16
NPBF16 = ml_dtypes.bfloat16

# ---- parity-class ordering ------------------------------------------------
def _class_order():
    order = []
    sizes = []
    for pi, pj in ((0, 0), (0, 1), (1, 0), (1, 1)):
        n0 = len(order)
        for i in range(pi, H, 2):
            for j in range(pj, H, 2):
                order.append(i * H + j)
        sizes.append(len(order) - n0)
    return order, sizes  # sizes = [16, 12, 12, 9]

ORDER, CSIZES = _class_order()
# chunks: (ij' slot range, kl' range) with <=128 partitions each
CHUNKS = [
    (0, 8, 0, 16),     # A1: 8 slots x 16 kl = 128
    (8, 16, 0, 16),    # A2: 128
    (16, 22, 16, 28),  # B1: 6 x 12 = 72
    (22, 28, 16, 28),  # B2: 72
    (28, 34, 28, 40),  # C1: 72
    (34, 40, 28, 40),  # C2: 72
    (40, 49, 40, 49),  # D : 9 x 9 = 81
]
NCHUNK = len(CHUNKS)
KTOT = sum((s1 - s0) * (k1 - k0) for s0, s1, k0, k1 in CHUNKS)  # 625

LAST_EXEC_NS = None
_CACHE = {}


# ---------------------------------------------------------------- device IR
def _build():
    dt = BF16
    nc = bass.Bass()

    x1h = nc.dram_tensor("x1h", [2, 128, BL * HW], dt, kind="ExternalInput")
    x2h = nc.dram_tensor("x2h", [2, 128, BL * HW], dt, kind="ExternalInput")
    w1h = nc.dram_tensor("w1h", [KTOT, REP], dt, kind="ExternalInput")
    w2h = nc.dram_tensor("w2h", [128, 64 * 128], dt, kind="ExternalInput")
    w3h = nc.dram_tensor("w3h", [128, 32], dt, kind="ExternalInput")
    b1h = nc.dram_tensor("b1h", [128, 8], F32, kind="ExternalInput")
    b2h = nc.dram_tensor("b2h", [128, 8], F32, kind="ExternalInput")
    b3h = nc.dram_tensor("b3h", [1, 4], dt, kind="ExternalInput")
    onesh = nc.dram_tensor("onesh", [1, 128], dt, kind="ExternalInput")
    outh = nc.dram_tensor("outh", [128, 4], F32, kind="ExternalOutput")

    with ExitStack() as ctx:
        sb = lambda name, shape, d: ctx.enter_context(nc.sbuf_tensor(name, shape, d))
        ps = lambda name, shape, d: ctx.enter_context(nc.psum_tensor(name, shape, d))
        sem = lambda name: ctx.enter_context(nc.semaphore(name))

        x1s = sb("x1s", [128, 2, BL * HW], dt)
        x2s = sb("x2s", [128, 2, BL * HW], dt)
        stage = sb("stage", [HW, HW, BL], dt)
        g7 = [
            sb(f"g{i}", [(s1 - s0) * (k1 - k0), BL], dt)
            for i, (s0, s1, k0, k1) in enumerate(CHUNKS)
        ]
        w7 = [
            sb(f"w{i}", [(s1 - s0) * (k1 - k0), REP], dt)
            for i, (s0, s1, k0, k1) in enumerate(CHUNKS)
        ]
        w2s = sb("w2s", [128, 8, 8, 128], dt)
        w3s = sb("w3s", [128, 8, 4], dt)
        b1T = sb("b1T", [128, 8], F32)
        b2T = sb("b2T", [128, 8], F32)
        b3s = sb("b3s", [1, 4], dt)
        ones = sb("ones", [1, 128], dt)
        relu1T = sb("relu1T", [128, 8, 128], dt)
        relu2T = sb("relu2T", [128, 8, 128], dt)
        outs = sb("outs", [128, 4], F32)

        psG = [ps(f"psG{i}", [HW, NG, HW], F32) for i in range(3)]
        psFT = ps("psFT", [128, 8, 128], F32)
        psO = ps("psO", [128, 4], F32)

        s_xq = [sem(f"s_xq{q}") for q in range(NQ)]  # per-quarter x DMAs
        s_w1 = sem("s_w1")    # 7 dmas -> 112
        s_w = sem("s_w")      # w2,w3 -> 32
        s_wb = sem("s_wb")    # b1,b2,b3,ones (ACT queue) -> 64
        s_g = sem("s_g")      # gram groups
        s_e = sem("s_e")      # evicts
        s_stk = sem("s_stk")  # stacking dmas -> 112
        s_f1 = sem("s_f1")
        s_r1 = sem("s_r1")
        s_f2 = sem("s_f2")
        s_r2 = sem("s_r2")
        s_f3 = sem("s_f3")
        s_oe = sem("s_oe")
        s_o = sem("s_o")

        block = ctx.enter_context(nc.Block())

        # ---------------- SP: x + weight DMAs, 3 stack DMAs, output
        @block.sync
        def _(sp):
            for q in range(NQ):
                lo, hi = q * QW, (q + 1) * QW
                for t in range(2):
                    sp.dma_start(x1s[:, t, lo:hi], x1h[t, :, lo:hi]).then_inc(s_xq[q], 16)
                    sp.dma_start(x2s[:, t, lo:hi], x2h[t, :, lo:hi]).then_inc(s_xq[q], 16)
            # stacking chunks 0..3 (ACT does 4..6); W2 et al AFTER so the
            # stack transfers aren't queued behind 2MB of fc2 weights
            sp.wait_ge(s_e, NGROUP)
            for i in (0, 1, 2, 3):
                s0, s1, k0, k1 = CHUNKS[i]
                sp.dma_start(g7[i][:, :], stage[s0:s1, k0:k1, :]).then_inc(s_stk, 16)
            sp.dma_start(
                w2s[:, :, :, :], w2h[:, :].rearrange("p (a b c) -> p a b c", a=8, b=8)
            ).then_inc(s_w, 16)
            sp.dma_start(
                w3s[:, :, :], w3h[:, :].rearrange("p (a b) -> p a b", a=8)
            ).then_inc(s_w, 16)
            sp.wait_ge(s_oe, 1)
            sp.dma_start(outh[:, :], outs[:, :]).then_inc(s_o, 16)
            sp.wait_ge(s_o, 16)

        # ---------------- PE
        @block.tensor
        def _(pe):
            # Gram: psG[ij', bb, kl'] per RoI (lhsT = x1 so partitions = ij)
            for gi in range(NGROUP):
                if gi % (NGROUP // NQ) == 0:
                    q = gi // (NGROUP // NQ)
                    pe.wait_ge(s_xq[q], 64)
                if gi >= 3:
                    pe.wait_ge(s_e, gi - 2)
                for bb in range(NG):
                    lb = gi * NG + bb
                    for t in range(2):
                        mm = pe.matmul(
                            psG[gi % 3][:, bb, :],
                            x1s[:, t, lb * HW : (lb + 1) * HW],
                            x2s[:, t, lb * HW : (lb + 1) * HW],
                            start=(t == 0),
                            stop=(t == 1),
                        )
                mm.then_inc(s_g, 1)

            # fc1: psFT[r, k, b] += w7[c][:, kchunk]^T @ g7[c]
            pe.wait_ge(s_stk, 112)
            pe.wait_ge(s_w1, 112)
            for k in range(8):
                for c in range(NCHUNK):
                    mm = pe.matmul(
                        psFT[:, k, :],
                        w7[c][:, k * 128 : (k + 1) * 128],
                        g7[c][:, :],
                        start=(c == 0),
                        stop=(c == NCHUNK - 1),
                    )
                mm.then_inc(s_f1, 1)

            # fc2: psFT reused; wait all relu1 evictions
            pe.wait_ge(s_r1, 8)
            pe.wait_ge(s_w, 32)
            for m in range(8):
                for kk in range(8):
                    mm = pe.matmul(
                        psFT[:, m, :],
                        w2s[:, kk, m, :],
                        relu1T[:, kk, :],
                        start=(kk == 0),
                        stop=(kk == 7),
                    )
                mm.then_inc(s_f2, 1)

            # fc3
            for m in range(8):
                pe.wait_ge(s_r2, m + 1)
                pe.matmul(
                    psO[:, :],
                    relu2T[:, m, :],
                    w3s[:, m, :],
                    start=(m == 0),
                    stop=False,
                )
            pe.wait_ge(s_wb, 64)
            pe.matmul(psO[:, :], ones[:, :], b3s[:, :], start=False, stop=True).then_inc(
                s_f3, 1
            )

        # ---------------- ACT: evictions + ReLUs + out copy
        @block.scalar
        def _(act):
            # W1 chunks + small biases on ACT's DMA queue: land early,
            # bandwidth-shared with x
            off = 0
            for i, (s0, s1, k0, k1) in enumerate(CHUNKS):
                kk = (s1 - s0) * (k1 - k0)
                act.dma_start(w7[i][:, :], w1h[off : off + kk, :]).then_inc(s_w1, 16)
                off += kk
            act.dma_start(b1T[:, :], b1h[:, :]).then_inc(s_wb, 16)
            act.dma_start(b2T[:, :], b2h[:, :]).then_inc(s_wb, 16)
            act.dma_start(b3s[:, :], b3h[:, :]).then_inc(s_wb, 16)
            act.dma_start(ones[:, :], onesh[:, :]).then_inc(s_wb, 16)
            for gi in range(NGROUP):
                act.wait_ge(s_g, gi + 1)
                act.activation(
                    stage[:, :, gi * NG : (gi + 1) * NG],
                    psG[gi % 3][:, :, :].rearrange("p b i -> p i b"),
                    mybir.ActivationFunctionType.Copy,
                ).then_inc(s_e, 1)
            # stacking chunks 4..6
            act.wait_ge(s_e, NGROUP)
            for i in (4, 5, 6):
                s0, s1, k0, k1 = CHUNKS[i]
                act.dma_start(g7[i][:, :], stage[s0:s1, k0:k1, :]).then_inc(s_stk, 16)
            # bank-granular: evict 4 chunks (one 2KB psum region) at a time so
            # no read overlaps an open accumulation group in the same bank
            act.wait_ge(s_wb, 64)
            for k in range(8):
                if k % 4 == 0:
                    act.wait_ge(s_f1, k + 4)
                act.activation(
                    relu1T[:, k, :],
                    psFT[:, k, :],
                    mybir.ActivationFunctionType.Relu,
                    bias=b1T[:, k : k + 1],
                ).then_inc(s_r1, 1)
            for m in range(8):
                if m % 4 == 0:
                    act.wait_ge(s_f2, m + 4)
                act.activation(
                    relu2T[:, m, :],
                    psFT[:, m, :],
                    mybir.ActivationFunctionType.Relu,
                    bias=b2T[:, m : m + 1],
                ).then_inc(s_r2, 1)
            act.wait_ge(s_f3, 1)
            act.activation(
                outs[:, :], psO[:, :], mybir.ActivationFunctionType.Copy
            ).then_inc(s_oe, 1)

    return nc


def _get_nc():
    if "nc" not in _CACHE:
        _CACHE["nc"] = _build()
    return _CACHE["nc"]


# ---------------------------------------------------------------- host prep
def _w1_packed(W1):
    """[625, 1024] rows = (chunk, slot, kl) in CHUNKS order."""
    out = np.zeros((KTOT, REP), dtype=np.float32)
    r = 0
    for s0, s1, k0, k1 in CHUNKS:
        for sl in range(s0, s1):
            ij = ORDER[sl]
            i, j = divmod(ij, H)
            for kx in range(k0, k1):
                kl = ORDER[kx]
                k, l = divmod(kl, H)
                ph = (k - i) // 2 + 7
                pw = (l - j) // 2 + 7
                f = (ph * P + pw) * HW + ij
                out[r, :] = W1[:, f]
                r += 1
    assert r == KTOT
    return out


# ---------------------------------------------------------------- entry
def kernel(patch1, patch2, W1, b1, W2, b2, W3, b3):
    global LAST_EXEC_NS

    patch1 = np.asarray(patch1, dtype=np.float32).reshape(B, C, HW)[:, :, ORDER]
    patch2 = np.asarray(patch2, dtype=np.float32).reshape(B, C, HW)[:, :, ORDER]
    W1 = np.asarray(W1, dtype=np.float32)
    W2 = np.asarray(W2, dtype=np.float32)
    W3 = np.asarray(W3, dtype=np.float32)
    b1 = np.asarray(b1, dtype=np.float32)
    b2 = np.asarray(b2, dtype=np.float32)
    b3 = np.asarray(b3, dtype=np.float32)

    w1p = _w1_packed(W1).astype(NPBF16)
    w2e = np.ascontiguousarray(
        W2.T.reshape(8, 128, 8, 128).transpose(1, 0, 2, 3).reshape(128, 64 * 128)
    ).astype(NPBF16)
    w3e = np.ascontiguousarray(
        W3.T.reshape(8, 128, 4).transpose(1, 0, 2).reshape(128, 32)
    ).astype(NPBF16)

    shared = {
        "w1h": w1p,
        "w2h": w2e,
        "w3h": w3e,
        "b1h": np.ascontiguousarray(b1.reshape(8, 128).T),
        "b2h": np.ascontiguousarray(b2.reshape(8, 128).T),
        "b3h": b3.reshape(1, 4).astype(NPBF16),
        "onesh": np.ones((1, 128), dtype=NPBF16),
    }

    in_maps = []
    for i in range(N_CORES):
        sl = slice(i * BL, (i + 1) * BL)
        x1 = np.ascontiguousarray(
            patch1[sl].reshape(BL, 2, 128, HW).transpose(1, 2, 0, 3).reshape(2, 128, BL * HW)
        ).astype(NPBF16)
        x2 = np.ascontiguousarray(
            patch2[sl].reshape(BL, 2, 128, HW).transpose(1, 2, 0, 3).reshape(2, 128, BL * HW)
        ).astype(NPBF16)
        in_maps.append({"x1h": x1, "x2h": x2, **shared})

    nc = _get_nc()
    trace = os.environ.get("CORR_TRACE", "0") == "1"
    res = run_bass_kernel_spmd(nc, in_maps, list(range(N_CORES)), trace=trace)
    LAST_EXEC_NS = res.exec_time_ns

    out = np.concatenate(
        [res.results[i]["outh"] for i in range(N_CORES)], axis=0
    ).astype(np.float32)
    return out


# revision 13
# speedup vs baseline: 1.0884x; 1.0884x over previous
"""CorrelationHead Trainium2 kernel — V2 (parity-class packed fc1).

Math: corr features reduce to per-RoI Gram G[ij,kl] = sum_c x1[c,ij]*x2[c,kl];
only (ij,kl) with matching (i,k) and (j,l) parity are used by fc1 (625 of
2401). Spatial positions are CLASS-SORTED on host (parity classes A=16/B=12/
C=12/D=9 for both ij and kl), so the Gram in SBUF is [49 ij', 49 kl', b] with
class-contiguous ranges. SBUF->SBUF DMAs stack (slot,kl) pairs onto partitions
(7 chunks of <=128 rows = 625 valid pairs) and fc1 becomes 7 accumulating
matmuls per rep-chunk (K=128/72/81) instead of 49 K=49 matmuls.

fc1/fc2 computed transposed (psFT[r, b]) so ReLU bias rides the ACT bias port
(per-partition = per-rep) and no PE transposes are needed anywhere.

Sharding: pure data-parallel, 128 RoIs per core, weights replicated.
"""

import os
from contextlib import ExitStack

import numpy as np
import ml_dtypes

import concourse.bass as bass
import concourse.mybir as mybir
from concourse.bass_utils import run_bass_kernel_spmd

P = 16
H = 7
C = 256
B = 1024
REP = 1024
HW = H * H
N_CORES = 8
BL = B // N_CORES          # 128 RoIs per core
NG = 8                     # RoIs per psG group
NGROUP = BL // NG          # 16
NQ = 4                     # x DMA quarters
QW = BL // NQ * HW         # 1568 cols per quarter

F32 = mybir.dt.float32
BF16 = mybir.dt.bfloat16
NPBF16 = ml_dtypes.bfloat16

# ---- parity-class ordering ------------------------------------------------
def _class_order():
    order = []
    sizes = []
    for pi, pj in ((0, 0), (0, 1), (1, 0), (1, 1)):
        n0 = len(order)
        for i in range(pi, H, 2):
            for j in range(pj, H, 2):
                order.append(i * H + j)
        sizes.append(len(order) - n0)
    return order, sizes  # sizes = [16, 12, 12, 9]

ORDER, CSIZES = _class_order()
# chunks: (ij' slot range, kl' range) with <=128 partitions each
CHUNKS = [
    (0, 8, 0, 16),     # A1: 8 slots x 16 kl = 128
    (8, 16, 0, 16),    # A2: 128
    (16, 22, 16, 28),  # B1: 6 x 12 = 72
    (22, 28, 16, 28),  # B2: 72
    (28, 34, 28, 40),  # C1: 72
    (34, 40, 28, 40),  # C2: 72
    (40, 49, 40, 49),  # D : 9 x 9 = 81
]
NCHUNK = len(CHUNKS)
KTOT = sum((s1 - s0) * (k1 - k0) for s0, s1, k0, k1 in CHUNKS)  # 625

LAST_EXEC_NS = None
_CACHE = {}


# ---------------------------------------------------------------- device IR
def _build():
    dt = BF16
    nc = bass.Bass()

    x1h = nc.dram_tensor("x1h", [2, 128, BL * HW], dt, kind="ExternalInput")
    x2h = nc.dram_tensor("x2h", [2, 128, BL * HW], dt, kind="ExternalInput")
    w1h = nc.dram_tensor("w1h", [KTOT, REP], dt, kind="ExternalInput")
    w2h = nc.dram_tensor("w2h", [128, 64 * 128], dt, kind="ExternalInput")
    w3h = nc.dram_tensor("w3h", [128, 32], dt, kind="ExternalInput")
    b1h = nc.dram_tensor("b1h", [128, 8], F32, kind="ExternalInput")
    b2h = nc.dram_tensor("b2h", [128, 8], F32, kind="ExternalInput")
    b3h = nc.dram_tensor("b3h", [1, 4], dt, kind="ExternalInput")
    onesh = nc.dram_tensor("onesh", [1, 128], dt, kind="ExternalInput")
    outh = nc.dram_tensor("outh", [128, 4], F32, kind="ExternalOutput")

    with ExitStack() as ctx:
        sb = lambda name, shape, d: ctx.enter_context(nc.sbuf_tensor(name, shape, d))
        ps = lambda name, shape, d: ctx.enter_context(nc.psum_tensor(name, shape, d))
        sem = lambda name: ctx.enter_context(nc.semaphore(name))

        x1s = sb("x1s", [128, 2, BL * HW], dt)
        x2s = sb("x2s", [128, 2, BL * HW], dt)
        stage = sb("stage", [HW, HW, BL], dt)
        g7 = [
            sb(f"g{i}", [(s1 - s0) * (k1 - k0), BL], dt)
            for i, (s0, s1, k0, k1) in enumerate(CHUNKS)
        ]
        w7 = [
            sb(f"w{i}", [(s1 - s0) * (k1 - k0), REP], dt)
            for i, (s0, s1, k0, k1) in enumerate(CHUNKS)
        ]
        w2s = sb("w2s", [128, 8, 8, 128], dt)
        w3s = sb("w3s", [128, 8, 4], dt)
        b1T = sb("b1T", [128, 8], F32)
        b2T = sb("b2T", [128, 8], F32)
        b3s = sb("b3s", [1, 4], dt)
        ones = sb("ones", [1, 128], dt)
        relu1T = sb("relu1T", [128, 8, 128], dt)
        relu2T = sb("relu2T", [128, 8, 128], dt)
        outs = sb("outs", [128, 4], F32)

        psG = [ps(f"psG{i}", [HW, NG, HW], F32) for i in range(3)]
        psFT = ps("psFT", [128, 8, 128], F32)
        psO = ps("psO", [128, 4], F32)

        s_xq = [sem(f"s_xq{q}") for q in range(NQ)]  # per-quarter x DMAs
        s_w1 = sem("s_w1")    # 7 dmas -> 112
        s_w = sem("s_w")      # w2,w3 -> 32
        s_wb = sem("s_wb")    # b1,b2,b3,ones (ACT queue) -> 64
        s_g = sem("s_g")      # gram groups
        s_e = sem("s_e")      # evicts
        s_stk = sem("s_stk")  # stacking dmas -> 112
        s_f1 = sem("s_f1")
        s_r1 = sem("s_r1")
        s_f2 = sem("s_f2")
        s_r2 = sem("s_r2")
        s_f3 = sem("s_f3")
        s_oe = sem("s_oe")
        s_o = sem("s_o")

        block = ctx.enter_context(nc.Block())

        # ---------------- SP: x + weight DMAs, 3 stack DMAs, output
        @block.sync
        def _(sp):
            for q in range(NQ):
                lo, hi = q * QW, (q + 1) * QW
                for t in range(2):
                    sp.dma_start(x1s[:, t, lo:hi], x1h[t, :, lo:hi]).then_inc(s_xq[q], 16)
                    sp.dma_start(x2s[:, t, lo:hi], x2h[t, :, lo:hi]).then_inc(s_xq[q], 16)
            # W1 right behind x on the same queue: x keeps full bandwidth,
            # W1 lands just before the stacks complete
            off = 0
            for i, (s0, s1, k0, k1) in enumerate(CHUNKS):
                kk = (s1 - s0) * (k1 - k0)
                sp.dma_start(w7[i][:, :], w1h[off : off + kk, :]).then_inc(s_w1, 16)
                off += kk
            # stacking chunks 0..3 (ACT does 4..6); W2 et al AFTER so the
            # stack transfers aren't queued behind 2MB of fc2 weights
            sp.wait_ge(s_e, NGROUP)
            for i in (0, 1, 2, 3):
                s0, s1, k0, k1 = CHUNKS[i]
                sp.dma_start(g7[i][:, :], stage[s0:s1, k0:k1, :]).then_inc(s_stk, 16)
            sp.dma_start(
                w2s[:, :, :, :], w2h[:, :].rearrange("p (a b c) -> p a b c", a=8, b=8)
            ).then_inc(s_w, 16)
            sp.dma_start(
                w3s[:, :, :], w3h[:, :].rearrange("p (a b) -> p a b", a=8)
            ).then_inc(s_w, 16)
            sp.wait_ge(s_oe, 1)
            sp.dma_start(outh[:, :], outs[:, :]).then_inc(s_o, 16)
            sp.wait_ge(s_o, 16)

        # ---------------- PE
        @block.tensor
        def _(pe):
            # Gram: psG[ij', bb, kl'] per RoI (lhsT = x1 so partitions = ij)
            for gi in range(NGROUP):
                if gi % (NGROUP // NQ) == 0:
                    q = gi // (NGROUP // NQ)
                    pe.wait_ge(s_xq[q], 64)
                if gi >= 3:
                    pe.wait_ge(s_e, gi - 2)
                for bb in range(NG):
                    lb = gi * NG + bb
                    for t in range(2):
                        mm = pe.matmul(
                            psG[gi % 3][:, bb, :],
                            x1s[:, t, lb * HW : (lb + 1) * HW],
                            x2s[:, t, lb * HW : (lb + 1) * HW],
                            start=(t == 0),
                            stop=(t == 1),
                        )
                mm.then_inc(s_g, 1)

            # fc1: psFT[r, k, b] += w7[c][:, kchunk]^T @ g7[c]
            pe.wait_ge(s_stk, 112)
            pe.wait_ge(s_w1, 112)
            for k in range(8):
                for c in range(NCHUNK):
                    mm = pe.matmul(
                        psFT[:, k, :],
                        w7[c][:, k * 128 : (k + 1) * 128],
                        g7[c][:, :],
                        start=(c == 0),
                        stop=(c == NCHUNK - 1),
                    )
                mm.then_inc(s_f1, 1)

            # fc2: psFT reused; wait all relu1 evictions
            pe.wait_ge(s_r1, 8)
            pe.wait_ge(s_w, 32)
            for m in range(8):
                for kk in range(8):
                    mm = pe.matmul(
                        psFT[:, m, :],
                        w2s[:, kk, m, :],
                        relu1T[:, kk, :],
                        start=(kk == 0),
                        stop=(kk == 7),
                    )
                mm.then_inc(s_f2, 1)

            # fc3
            for m in range(8):
                pe.wait_ge(s_r2, m + 1)
                pe.matmul(
                    psO[:, :],
                    relu2T[:, m, :],
                    w3s[:, m, :],
                    start=(m == 0),
                    stop=False,
                )
            pe.wait_ge(s_wb, 64)
            pe.matmul(psO[:, :], ones[:, :], b3s[:, :], start=False, stop=True).then_inc(
                s_f3, 1
            )

        # ---------------- ACT: evictions + ReLUs + out copy
        @block.scalar
        def _(act):
            # small biases on ACT's otherwise-empty DMA queue
            act.dma_start(b1T[:, :], b1h[:, :]).then_inc(s_wb, 16)
            act.dma_start(b2T[:, :], b2h[:, :]).then_inc(s_wb, 16)
            act.dma_start(b3s[:, :], b3h[:, :]).then_inc(s_wb, 16)
            act.dma_start(ones[:, :], onesh[:, :]).then_inc(s_wb, 16)
            for gi in range(NGROUP):
                act.wait_ge(s_g, gi + 1)
                act.activation(
                    stage[:, :, gi * NG : (gi + 1) * NG],
                    psG[gi % 3][:, :, :].rearrange("p b i -> p i b"),
                    mybir.ActivationFunctionType.Copy,
                ).then_inc(s_e, 1)
            # stacking chunks 4..6
            act.wait_ge(s_e, NGROUP)
            for i in (4, 5, 6):
                s0, s1, k0, k1 = CHUNKS[i]
                act.dma_start(g7[i][:, :], stage[s0:s1, k0:k1, :]).then_inc(s_stk, 16)
            # bank-granular: evict 4 chunks (one 2KB psum region) at a time so
            # no read overlaps an open accumulation group in the same bank
            act.wait_ge(s_wb, 64)
            for k in range(8):
                if k % 4 == 0:
                    act.wait_ge(s_f1, k + 4)
                act.activation(
                    relu1T[:, k, :],
                    psFT[:, k, :],
                    mybir.ActivationFunctionType.Relu,
                    bias=b1T[:, k : k + 1],
                ).then_inc(s_r1, 1)
            for m in range(8):
                if m % 4 == 0:
                    act.wait_ge(s_f2, m + 4)
                act.activation(
                    relu2T[:, m, :],
                    psFT[:, m, :],
                    mybir.ActivationFunctionType.Relu,
                    bias=b2T[:, m : m + 1],
                ).then_inc(s_r2, 1)
            act.wait_ge(s_f3, 1)
            act.activation(
                outs[:, :], psO[:, :], mybir.ActivationFunctionType.Copy
            ).then_inc(s_oe, 1)

    return nc


def _get_nc():
    if "nc" not in _CACHE:
        _CACHE["nc"] = _build()
    return _CACHE["nc"]


# ---------------------------------------------------------------- host prep
def _w1_packed(W1):
    """[625, 1024] rows = (chunk, slot, kl) in CHUNKS order."""
    out = np.zeros((KTOT, REP), dtype=np.float32)
    r = 0
    for s0, s1, k0, k1 in CHUNKS:
        for sl in range(s0, s1):
            ij = ORDER[sl]
            i, j = divmod(ij, H)
            for kx in range(k0, k1):
                kl = ORDER[kx]
                k, l = divmod(kl, H)
                ph = (k - i) // 2 + 7
                pw = (l - j) // 2 + 7
                f = (ph * P + pw) * HW + ij
                out[r, :] = W1[:, f]
                r += 1
    assert r == KTOT
    return out


# ---------------------------------------------------------------- entry
def kernel(patch1, patch2, W1, b1, W2, b2, W3, b3):
    global LAST_EXEC_NS

    patch1 = np.asarray(patch1, dtype=np.float32).reshape(B, C, HW)[:, :, ORDER]
    patch2 = np.asarray(patch2, dtype=np.float32).reshape(B, C, HW)[:, :, ORDER]
    W1 = np.asarray(W1, dtype=np.float32)
    W2 = np.asarray(W2, dtype=np.float32)
    W3 = np.asarray(W3, dtype=np.float32)
    b1 = np.asarray(b1, dtype=np.float32)
    b2 = np.asarray(b2, dtype=np.float32)
    b3 = np.asarray(b3, dtype=np.float32)

    w1p = _w1_packed(W1).astype(NPBF16)
    w2e = np.ascontiguousarray(
        W2.T.reshape(8, 128, 8, 128).transpose(1, 0, 2, 3).reshape(128, 64 * 128)
    ).astype(NPBF16)
    w3e = np.ascontiguousarray(
        W3.T.reshape(8, 128, 4).transpose(1, 0, 2).reshape(128, 32)
    ).astype(NPBF16)

    shared = {
        "w1h": w1p,
        "w2h": w2e,
        "w3h": w3e,
        "b1h": np.ascontiguousarray(b1.reshape(8, 128).T),
        "b2h": np.ascontiguousarray(b2.reshape(8, 128).T),
        "b3h": b3.reshape(1, 4).astype(NPBF16),
        "onesh": np.ones((1, 128), dtype=NPBF16),
    }

    in_maps = []
    for i in range(N_CORES):
        sl = slice(i * BL, (i + 1) * BL)
        x1 = np.ascontiguousarray(
            patch1[sl].reshape(BL, 2, 128, HW).transpose(1, 2, 0, 3).reshape(2, 128, BL * HW)
        ).astype(NPBF16)
        x2 = np.ascontiguousarray(
            patch2[sl].reshape(BL, 2, 128, HW).transpose(1, 2, 0, 3).reshape(2, 128, BL * HW)
        ).astype(NPBF16)
        in_maps.append({"x1h": x1, "x2h": x2, **shared})

    nc = _get_nc()
    trace = os.environ.get("CORR_TRACE", "0") == "1"
    res = run_bass_kernel_spmd(nc, in_maps, list(range(N_CORES)), trace=trace)
    LAST_EXEC_NS = res.exec_time_ns

    out = np.concatenate(
        [res.results[i]["outh"] for i in range(N_CORES)], axis=0
    ).astype(np.float32)
    return out
